# revision 1
# baseline (speedup 1.0000x reference)
"""LATTE GNN message passing on 8 trn2 cores.

Design (v2):
- Edges sharded by dst node: core k owns dst nodes [6272k, 6272(k+1)).
- Stage 1: h = relu(feats @ W) and per-node attention dots (aj, ai) for all
  50176 nodes, written to a DRAM gather table TBL (bf16, 512B rows,
  row = node+1; rows 0 / 50177 are pad rows with h=0, aj=-100).
- Stage 1o: same projection recomputed for the core's own 6272 nodes from a
  per-core feats slice into OWNT (keeps the instruction stream core-uniform
  across the 8 SPMD cores).
- Stage 2: 8 passes (metapath x src-chunk). Within a pass, each edge sits in
  the partition lane of its dst node; lanes are packed into 49 blocks of 128
  by descending degree so per-block tile counts stay near the mean
  (uniformized across cores). h[src] rows are fetched with batched
  dma_gather (1024 rows/instr, 4 SWDGE queues round-robin; the int16 index
  limit is why src ids are split into chunks A = rows 0..32767 and
  B = rows 32768..50177). Per-edge softmax weights
  s = exp(leakyrelu(ai[dst]+aj[src])) are computed on DVE/ACT; messages h*s
  are summed over the free axis with a binary tree of DVE adds (no PE
  scatter matmuls). Partial sums + s-sums land in ERAW per pass.
- Stage 2b: dma_gather realigns both chunk passes of each metapath back to
  original node order and adds them -> EALN.
- Stage 3: per 128-node block: normalize by s-sums, beta relation attention
  (softmax over 5 relations), weighted sum, relu -> OUT.

Pad slots gather the pad rows whose h=0 / aj=-100 make their contribution
negligible (~e^-20); the host reorders nothing on device - only the final
OUT rows are already in original node order.
"""

import sys
import numpy as np

sys.path.insert(0, "/root/shadow")
try:
    import setup_ntff  # noqa: F401
except Exception:
    pass
sys.path.insert(0, "/opt/trn_rl_repo")

import ml_dtypes
import concourse.bass as bass
import concourse.bacc as bacc
import concourse.mybir as mybir
import concourse.tile as tile
from concourse.bass_utils import run_bass_kernel_spmd
from concourse.library_config import mlp

N = 50000
E = 800000
M = 4
NP = 2 * M               # passes: (m, chunk)
H = 4
C = 32
IN = 256
D = 128
NCORES = 8
CORE_N = 6272
NB = 49
NT = 50176
ROWS = NT + 2            # + pad rows 0 and 50177
CHA = 32767              # nodes 0..32766 -> chunk A (row = node+1)
NBROWS = ROWS - 32768    # 17410 chunk-B rows
PADB = NBROWS - 1        # chunk-B-local pad idx (row 50177)
GSZ = 1024               # slots per dma_gather (ucode ring limit)
GT = GSZ // 128
GW = GSZ // 16
EW = 256                 # table row: 256 bf16 = 512B
F32 = mybir.dt.float32
BF16 = mybir.dt.bfloat16
I16 = mybir.dt.int16
AF = mybir.ActivationFunctionType
OP = mybir.AluOpType

_CACHE = {}


# ---------------------------------------------------------------- host prep

def _wrap_chunks(stream):
    """idx stream (len multiple of 16) -> wrapped [128, len/16] int16.
    dma_gather reads idx j from (partition j%16, col j//16), replicated
    across the 8 groups of 16 partitions."""
    n = len(stream)
    a = np.asarray(stream, dtype=np.int16).reshape(n // 16, 16).T
    return np.tile(a, (8, 1))


def _plan_and_pack(edge_index):
    ei = np.asarray(edge_index)
    perms = {}
    ranks = {}
    edges = {}
    Tk = np.zeros((NCORES, NP, NB), dtype=np.int64)
    for k in range(NCORES):
        for m in range(M):
            src = ei[m, 0].astype(np.int64)
            dst = ei[m, 1].astype(np.int64)
            sel = (dst // CORE_N) == k
            ls = dst[sel] - CORE_N * k
            sr = src[sel]
            isA = sr <= CHA - 1
            for P, mask in ((2 * m, isA), (2 * m + 1, ~isA)):
                lw, sw = ls[mask], sr[mask]
                deg = np.bincount(lw, minlength=CORE_N)
                perm = np.argsort(-deg, kind="stable")
                rank = np.empty(CORE_N, dtype=np.int64)
                rank[perm] = np.arange(CORE_N)
                perms[(k, P)] = perm
                ranks[(k, P)] = rank
                edges[(k, P)] = (lw, sw, rank)
                Tk[k, P] = deg[perm].reshape(NB, 128).max(axis=1)

    Tu = Tk.max(axis=0)                       # [NP, NB]
    cum = np.concatenate(
        [np.zeros((NP, 1), np.int64), np.cumsum(Tu, axis=1)], axis=1)
    nch = [int(-(-cum[P, -1] * 128 // GSZ)) for P in range(NP)]
    plan = {"Tu": Tu, "cum": cum, "nch": nch, "nch_own": -(-CORE_N // GSZ)}

    percore = []
    for k in range(NCORES):
        cols = []
        for P in range(NP):
            lw, sw, rank = edges[(k, P)]
            lane = rank[lw]
            order = np.argsort(lane, kind="stable")
            lw2, sw2 = lane[order], sw[order]
            first = np.searchsorted(lw2, lw2)
            occ = np.arange(len(lw2)) - first
            b = lw2 // 128
            p = lw2 % 128
            slot = (cum[P, b] + occ) * 128 + p
            S = nch[P] * GSZ
            if P % 2 == 0:
                stream = np.zeros(S, dtype=np.int64)
                stream[slot] = sw2 + 1
            else:
                stream = np.full(S, PADB, dtype=np.int64)
                stream[slot] = sw2 - CHA
            cols.append(_wrap_chunks(stream))
        for P in range(NP):
            s = np.zeros(plan["nch_own"] * GSZ, dtype=np.int64)
            s[:CORE_N] = ranks[(k, P)]
            cols.append(_wrap_chunks(s))
        percore.append(np.ascontiguousarray(np.concatenate(cols, axis=1)))
    plan["idx_cols"] = percore[0].shape[1]
    plan["perms"] = perms
    return plan, percore


def _prep_host(feats, edge_index, W, attn, rel_attn_l, rel_attn_r,
               rel_attn_bias):
    plan, idxw = _plan_and_pack(edge_index)

    featsT = np.zeros((IN, NT), dtype=np.float32)
    featsT[:, :N] = np.asarray(feats, dtype=np.float32).T
    featsT = featsT.astype(ml_dtypes.bfloat16)
    Wb = np.asarray(W, dtype=np.float32).astype(ml_dtypes.bfloat16)
    attn = np.asarray(attn, dtype=np.float32)
    Acat = np.zeros((D, 32), dtype=np.float32)
    for m in range(M):
        for h in range(H):
            Acat[h * C:(h + 1) * C, m * H + h] = attn[m, h, C:]       # aj
            Acat[h * C:(h + 1) * C, 16 + m * H + h] = attn[m, h, :C]  # ai
    Acat = Acat.astype(ml_dtypes.bfloat16)
    ident = np.eye(128, dtype=np.float32).astype(ml_dtypes.bfloat16)
    padrow = np.zeros((1, EW), dtype=np.float32)
    padrow[0, 128:144] = -100.0
    padrow = padrow.astype(ml_dtypes.bfloat16)
    rel_l = np.asarray(rel_attn_l, dtype=np.float32).reshape(1, 128)
    rel_r = np.asarray(rel_attn_r, dtype=np.float32).reshape(1, 640)
    relb = np.repeat(np.asarray(rel_attn_bias, np.float32), 4).reshape(1, 20)
    zer = np.zeros((1, 132), dtype=np.float32)

    ft = np.asarray(feats, np.float32).T
    in_maps = []
    for k in range(NCORES):
        fown = np.zeros((IN, CORE_N), dtype=np.float32)
        lo = CORE_N * k
        hi = min(N, lo + CORE_N)
        fown[:, :hi - lo] = ft[:, lo:hi]
        fownp = np.empty((IN, NP * CORE_N), dtype=np.float32)
        for P in range(NP):
            fownp[:, P * CORE_N:(P + 1) * CORE_N] = \
                fown[:, plan["perms"][(k, P)]]
        in_maps.append({
            "featsT": featsT, "FOWN": fown.astype(ml_dtypes.bfloat16),
            "FOWNP": fownp.astype(ml_dtypes.bfloat16),
            "Wb": Wb, "ACAT": Acat, "IDENT": ident, "PADR": padrow,
            "RELL": rel_l, "RELR": rel_r, "RELB": relb, "ZER": zer,
            "IDXW": idxw[k],
        })
    return plan, in_maps


# ---------------------------------------------------------------- device

def _build(plan):
    nc = bacc.Bacc("TRN2", target_bir_lowering=False, debug=False,
                   num_swdge_queues=4)
    ds = bass.ds

    featsT = nc.dram_tensor("featsT", [IN, NT], BF16, kind="ExternalInput")
    FOWN = nc.dram_tensor("FOWN", [IN, CORE_N], BF16, kind="ExternalInput")
    FOWNP = nc.dram_tensor("FOWNP", [IN, NP * CORE_N], BF16,
                           kind="ExternalInput")
    Wb = nc.dram_tensor("Wb", [IN, D], BF16, kind="ExternalInput")
    ACAT = nc.dram_tensor("ACAT", [D, 32], BF16, kind="ExternalInput")
    IDENT = nc.dram_tensor("IDENT", [128, 128], BF16, kind="ExternalInput")
    PADR = nc.dram_tensor("PADR", [1, EW], BF16, kind="ExternalInput")
    RELL = nc.dram_tensor("RELL", [1, 128], F32, kind="ExternalInput")
    RELR = nc.dram_tensor("RELR", [1, 640], F32, kind="ExternalInput")
    RELB = nc.dram_tensor("RELB", [1, 20], F32, kind="ExternalInput")
    ZER = nc.dram_tensor("ZER", [1, 132], F32, kind="ExternalInput")
    IDXW = nc.dram_tensor("IDXW", [128, plan["idx_cols"]], I16,
                          kind="ExternalInput")

    TBA = nc.dram_tensor("TBA", [32768, EW], BF16)
    TBB = nc.dram_tensor("TBB", [NBROWS, EW], BF16)
    OWNT = nc.dram_tensor("OWNT", [CORE_N, EW], BF16)
    ERAW = nc.dram_tensor("ERAW", [NP * CORE_N, 192], F32)
    EALN = nc.dram_tensor("EALN", [CORE_N, 528], F32)
    OUT = nc.dram_tensor("OUT", [CORE_N, D], F32, kind="ExternalOutput")

    Tu, cum, nch = plan["Tu"], plan["cum"], plan["nch"]
    NCHO = plan["nch_own"]
    Tmax = int(Tu.max())

    qn = [0]

    def nextq():
        q = qn[0] % 4
        qn[0] += 1
        return q

    with tile.TileContext(nc) as tc:
        with tc.tile_pool(name="const", bufs=1) as cp:
            nc.gpsimd.load_library(mlp)
            W0 = cp.tile([128, 128], BF16)
            nc.sync.dma_start(out=W0[:], in_=Wb[0:128, :])
            W1 = cp.tile([128, 128], BF16)
            nc.sync.dma_start(out=W1[:], in_=Wb[128:256, :])
            Ac = cp.tile([128, 32], BF16)
            nc.sync.dma_start(out=Ac[:], in_=ACAT[:])
            idn = cp.tile([128, 128], BF16)
            nc.sync.dma_start(out=idn[:], in_=IDENT[:])
            rlr = cp.tile([128, 128], F32)
            nc.sync.dma_start(out=rlr[:], in_=RELL[:].to_broadcast((128, 128)))
            rrr = cp.tile([128, 640], F32)
            nc.sync.dma_start(out=rrr[:], in_=RELR[:].to_broadcast((128, 640)))
            rbb = cp.tile([128, 20], F32)
            nc.sync.dma_start(out=rbb[:], in_=RELB[:].to_broadcast((128, 20)))
            zrow = cp.tile([128, 132], F32)
            nc.sync.dma_start(out=zrow[:], in_=ZER[:].to_broadcast((128, 132)))
            rlrb = cp.tile([128, 128], BF16)
            nc.vector.tensor_copy(out=rlrb[:], in_=rlr[:])
            rrrb = cp.tile([128, 640], BF16)
            nc.vector.tensor_copy(out=rrrb[:], in_=rrr[:])
            pr0 = cp.tile([1, EW], BF16)
            nc.sync.dma_start(out=pr0[:], in_=PADR[:])
            nc.sync.dma_start(out=TBA[0:1, :], in_=pr0[:])
            nc.sync.dma_start(out=TBB[NBROWS - 1:NBROWS, :], in_=pr0[:])

            # ---- stage 1: projection ----
            def project(dst, src_dram, ncols, base_row, pool, psum,
                        dst2=None):
                nsteps = -(-ncols // 512)
                for i in range(nsteps):
                    w = min(512, ncols - i * 512)
                    ng = w // 128
                    fa = pool.tile([128, 512], BF16, tag="fa")
                    nc.sync.dma_start(out=fa[:, 0:w],
                                      in_=src_dram[0:128, ds(i * 512, w)])
                    fb = pool.tile([128, 512], BF16, tag="fb")
                    nc.sync.dma_start(out=fb[:, 0:w],
                                      in_=src_dram[128:256, ds(i * 512, w)])
                    hrow = pool.tile([128, 4 * 160], BF16, tag="hrow")
                    for j in range(ng):
                        hp = psum.tile([128, 128], F32, tag="hp")
                        nc.tensor.matmul(out=hp[:],
                                         lhsT=fa[:, j * 128:(j + 1) * 128],
                                         rhs=W0[:], start=True, stop=False)
                        nc.tensor.matmul(out=hp[:],
                                         lhsT=fb[:, j * 128:(j + 1) * 128],
                                         rhs=W1[:], start=False, stop=True)
                        nc.scalar.activation(
                            hrow[:, j * 160:j * 160 + 128], hp[:], AF.Relu)
                        htp = psum.tile([128, 128], BF16, tag="htp")
                        nc.tensor.transpose(
                            out=htp[:],
                            in_=hrow[:, j * 160:j * 160 + 128],
                            identity=idn[:])
                        hts = pool.tile([128, 128], BF16, tag="hts")
                        nc.vector.tensor_copy(out=hts[:], in_=htp[:])
                        ap2 = psum.tile([128, 32], F32, tag="ap2")
                        nc.tensor.matmul(out=ap2[:], lhsT=hts[:], rhs=Ac[:],
                                         start=True, stop=True)
                        nc.vector.tensor_copy(
                            out=hrow[:, j * 160 + 128:j * 160 + 160],
                            in_=ap2[:])
                    r0 = base_row + i * 512

                    def wout(tgt, row, g0, p0, np_, cnt=1):
                        nc.sync.dma_start(
                            out=tgt[ds(row, cnt * np_), 0:160].rearrange(
                                "(g p) c -> p g c", p=np_),
                            in_=hrow[p0:p0 + np_,
                                     g0 * 160:(g0 + cnt) * 160].rearrange(
                                "p (g c) -> p g c", c=160))

                    if dst2 is None or r0 + w <= 32768:
                        wout(dst, r0, 0, 0, 128, ng)
                    elif r0 >= 32768:
                        wout(dst2, r0 - 32768, 0, 0, 128, ng)
                    else:
                        nA = 32768 - r0
                        gA, pA = nA // 128, nA % 128
                        if gA > 0:
                            wout(dst, r0, 0, 0, 128, gA)
                        if pA > 0:
                            wout(dst, r0 + gA * 128, gA, 0, pA)
                            wout(dst2, 0, gA, pA, 128 - pA)
                        bstart = gA + (1 if pA else 0)
                        boff = bstart * 128 - nA
                        if bstart < ng:
                            wout(dst2, boff, bstart, 0, 128, ng - bstart)

            s2_cols = sum(nch[P] * GW for P in range(NP))
            aln_off = s2_cols

            def idx_ap(it, g):
                return it[:, g * GW:(g + 1) * GW]

            # stage 1o: own projection, then the global table projection.
            # ai[perm_P] is produced per pass by projecting host-permuted
            # own-features (PE is otherwise idle during stage 2).
            with tc.tile_pool(name="aib", bufs=1) as pa:
                with tc.tile_pool(name="s1", bufs=4) as p1, \
                     tc.tile_pool(name="s1p", bufs=2, space="PSUM") as pp1:
                    project(TBA, featsT, NT, 1, p1, pp1, dst2=TBB)
                    project(OWNT, FOWN, CORE_N, 0, p1, pp1)
                aibs = [pa.tile([128, NB * 4], BF16, tag=f"ab{P}",
                                name=f"ab{P}")
                        for P in range(NP)]

                def emit_aibs(P, pool, psum):
                    m = P // 2
                    ab = aibs[P]
                    base = P * CORE_N
                    nsteps = -(-CORE_N // 512)
                    for i in range(nsteps):
                        w = min(512, CORE_N - i * 512)
                        fa = pool.tile([128, 512], BF16, tag="pfa")
                        nc.sync.dma_start(
                            out=fa[:, 0:w],
                            in_=FOWNP[0:128, ds(base + i * 512, w)])
                        fb = pool.tile([128, 512], BF16, tag="pfb")
                        nc.sync.dma_start(
                            out=fb[:, 0:w],
                            in_=FOWNP[128:256, ds(base + i * 512, w)])
                        for j in range(w // 128):
                            hp = psum.tile([128, 128], F32, tag="php")
                            nc.tensor.matmul(
                                out=hp[:], lhsT=fa[:, j * 128:(j + 1) * 128],
                                rhs=W0[:], start=True, stop=False)
                            nc.tensor.matmul(
                                out=hp[:], lhsT=fb[:, j * 128:(j + 1) * 128],
                                rhs=W1[:], start=False, stop=True)
                            hrs = pool.tile([128, 128], BF16, tag="phr")
                            nc.scalar.activation(hrs[:], hp[:], AF.Relu)
                            htp = psum.tile([128, 128], BF16, tag="pht")
                            nc.tensor.transpose(out=htp[:], in_=hrs[:],
                                                identity=idn[:])
                            hts = pool.tile([128, 128], BF16, tag="pts")
                            nc.scalar.activation(hts[:], htp[:], AF.Copy)
                            ap4 = psum.tile([128, 4], F32, tag="pa4")
                            nc.tensor.matmul(
                                out=ap4[:], lhsT=hts[:],
                                rhs=Ac[:, 16 + 4 * m:20 + 4 * m],
                                start=True, stop=True)
                            b = i * 4 + j
                            nc.vector.tensor_copy(
                                out=ab[:, b * 4:(b + 1) * 4], in_=ap4[:])

                # ---- stage 2 ----
                scol = [0]
                nchmax = max(nch)
                with tc.tile_pool(name="idxs", bufs=1) as pix, \
                     tc.tile_pool(name="hgp", bufs=10) as pg, \
                     tc.tile_pool(name="msg", bufs=3) as pmsg, \
                     tc.tile_pool(name="sxp", bufs=2) as psxp, \
                     tc.tile_pool(name="rlp", bufs=2) as prl, \
                     tc.tile_pool(name="prj", bufs=3) as pprj, \
                     tc.tile_pool(name="prjp", bufs=2, space="PSUM") as pprp, \
                     tc.tile_pool(name="s3", bufs=2) as p3, \
                     tc.tile_pool(name="osb", bufs=3) as posb:

                    def emit_rl_chunk(m, P, g, ita, itb):
                        nbt = min(GT, NB - g * GT)
                        if nbt <= 0:
                            return
                        ra = prl.tile([128, GT * 192], F32, tag="ra",
                                      name="ra")
                        nc.gpsimd.dma_gather(
                            ra[:].rearrange("p (t e) -> p t e", e=192),
                            ERAW[ds((P - 1) * CORE_N, CORE_N), :],
                            idx_ap(ita, g), GSZ, GSZ, 192,
                            queue_num=nextq())
                        rb = prl.tile([128, GT * 192], F32, tag="rb",
                                      name="rb")
                        nc.gpsimd.dma_gather(
                            rb[:].rearrange("p (t e) -> p t e", e=192),
                            ERAW[ds(P * CORE_N, CORE_N), :],
                            idx_ap(itb, g), GSZ, GSZ, 192,
                            queue_num=nextq())
                        mgc = prl.tile([128, GT * 132], F32, tag="mgc",
                                       name="mgc")
                        nc.vector.tensor_tensor(
                            out=mgc[:, 0:nbt * 132].rearrange(
                                "p (t c) -> p t c", c=132),
                            in0=ra[:].rearrange("p (t e) -> p t e",
                                                e=192)[:, 0:nbt, 0:132],
                            in1=rb[:].rearrange("p (t e) -> p t e",
                                                e=192)[:, 0:nbt, 0:132],
                            op=OP.add)
                        nc.sync.dma_start(
                            out=EALN[ds(g * GSZ, nbt * 128),
                                     ds(m * 132, 132)].rearrange(
                                "(t p) c -> p t c", p=128),
                            in_=mgc[:, 0:nbt * 132].rearrange(
                                "p (t c) -> p t c", c=132))

                    def emit_s3(b):

                        eal = p3.tile([128, 528], F32, tag="eal")
                        nc.sync.dma_start(out=eal[:],
                                          in_=EALN[ds(b * 128, 128), :])
                        hb = p3.tile([128, 128], BF16, tag="hb")
                        nc.sync.dma_start(out=hb[:],
                                          in_=OWNT[ds(b * 128, 128), 0:128])
                        er5 = p3.tile([128, 640], BF16, tag="er5")
                        for m in range(M):
                            dn = p3.tile([128, 4], F32, tag="dn")
                            nc.vector.tensor_scalar_add(
                                out=dn[:],
                                in0=eal[:, m * 132 + 128:m * 132 + 132],
                                scalar1=1e-6)
                            rc = p3.tile([128, 4], F32, tag="rc")
                            nc.vector.reciprocal(out=rc[:], in_=dn[:])
                            nc.vector.tensor_tensor(
                                out=er5[:, m * 128:(m + 1) * 128].rearrange(
                                    "p (h c) -> p h c", c=32),
                                in0=eal[:, m * 132:m * 132 + 128].rearrange(
                                    "p (h c) -> p h c", c=32),
                                in1=rc[:].unsqueeze(2).to_broadcast((128, 4, 32)),
                                op=OP.mult)
                        nc.vector.tensor_copy(out=er5[:, 512:640], in_=hb[:])
                        bl0 = p3.tile([128, 128], BF16, tag="bl0")
                        nc.vector.tensor_tensor(out=bl0[:], in0=er5[:, 512:640],
                                                in1=rlrb[:], op=OP.mult)
                        blr = p3.tile([128, 128], BF16, tag="blr")
                        nc.scalar.activation(blr[:], bl0[:], AF.Relu)
                        t1 = p3.tile([128, 640], BF16, tag="t1")
                        nc.vector.tensor_tensor(out=t1[:], in0=er5[:],
                                                in1=rrrb[:], op=OP.mult)
                        t2 = p3.tile([128, 640], BF16, tag="t2")
                        nc.scalar.activation(t2[:], t1[:], AF.Relu)
                        t3 = p3.tile([128, 640], BF16, tag="t3")
                        nc.vector.tensor_tensor(
                            out=t3[:].rearrange("p (r d) -> p r d", d=128),
                            in0=t2[:].rearrange("p (r d) -> p r d", d=128),
                            in1=blr[:].unsqueeze(1).to_broadcast((128, 5, 128)),
                            op=OP.mult)
                        bmat = p3.tile([128, 20], F32, tag="bmat")
                        nc.vector.reduce_sum(
                            out=bmat[:],
                            in_=t3[:].rearrange("p (rh c) -> p rh c", c=32),
                            axis=mybir.AxisListType.X)
                        nc.vector.tensor_tensor(out=bmat[:], in0=bmat[:],
                                                in1=rbb[:], op=OP.add)
                        bview = bmat[:].rearrange("p (r h) -> p h r", h=4)
                        vmax = p3.tile([128, 4], F32, tag="vmax")
                        nc.vector.reduce_max(out=vmax[:], in_=bview,
                                             axis=mybir.AxisListType.X)
                        eb = p3.tile([128, 20], F32, tag="eb")
                        nc.vector.tensor_tensor(
                            out=eb[:].rearrange("p (r h) -> p h r", h=4),
                            in0=bview,
                            in1=vmax[:].unsqueeze(2).to_broadcast((128, 4, 5)),
                            op=OP.subtract)
                        eb2 = p3.tile([128, 20], F32, tag="eb2")
                        nc.scalar.activation(eb2[:], eb[:], AF.Exp)
                        vs = p3.tile([128, 4], F32, tag="vs")
                        nc.vector.reduce_sum(
                            out=vs[:],
                            in_=eb2[:].rearrange("p (r h) -> p h r", h=4),
                            axis=mybir.AxisListType.X)
                        rs = p3.tile([128, 4], F32, tag="rs")
                        nc.vector.reciprocal(out=rs[:], in_=vs[:])
                        bw = p3.tile([128, 20], BF16, tag="bw")
                        nc.vector.tensor_tensor(
                            out=bw[:].rearrange("p (r h) -> p h r", h=4),
                            in0=eb2[:].rearrange("p (r h) -> p h r", h=4),
                            in1=rs[:].unsqueeze(2).to_broadcast((128, 4, 5)),
                            op=OP.mult)
                        tm = p3.tile([128, 640], BF16, tag="tm")
                        nc.vector.tensor_tensor(
                            out=tm[:].rearrange("p (r h c) -> p r h c", h=4,
                                                c=32),
                            in0=er5[:].rearrange("p (r h c) -> p r h c", h=4,
                                                 c=32),
                            in1=bw[:].rearrange("p (r h) -> p r h", h=4
                                                ).unsqueeze(3).to_broadcast(
                                                    (128, 5, 4, 32)),
                            op=OP.mult)
                        acc = p3.tile([128, 128], F32, tag="acc")
                        nc.vector.reduce_sum(
                            out=acc[:],
                            in_=tm[:].rearrange("p (r d) -> p d r", d=128),
                            axis=mybir.AxisListType.X)
                        ob = p3.tile([128, 128], F32, tag="ob")
                        nc.scalar.activation(ob[:], acc[:], AF.Relu)
                        nc.sync.dma_start(out=OUT[ds(b * 128, 128), :], in_=ob[:])

                    for P in range(NP):
                        m = P // 2
                        emit_aibs(P, pprj, pprp)
                        offP = scol[0]
                        scol[0] += nch[P] * GW
                        ixf = pix.tile([128, nchmax * GW], I16, tag="ix")
                        ix = ixf[:, 0:nch[P] * GW]
                        nc.sync.dma_start(out=ix,
                                          in_=IDXW[:, ds(offP, nch[P] * GW)])
                        src = TBA[:] if P % 2 == 0 else TBB[:]

                        cache = {}
                        nxt = [0]

                        def need(upto_tile, cache=cache, nxt=nxt, ix=ix,
                                 src=src, nchP=nch[P]):
                            while nxt[0] * GT < upto_tile and nxt[0] < nchP:
                                g = nxt[0]
                                hg = pg.tile([128, GT * EW], BF16, tag="hg")
                                hv = hg[:].rearrange("p (t e) -> p t e", e=EW)
                                nc.gpsimd.dma_gather(
                                    hv[:, :, :], src, idx_ap(ix, g),
                                    GSZ, GSZ, EW, queue_num=nextq())
                                cache[g] = hv
                                nxt[0] += 1

                        GRP = 8
                        osbw = None
                        for b in range(NB):
                            if b % GRP == 0:
                                osbw = posb.tile([128, 8 * 132], F32,
                                                 tag="osbw")
                            osb = osbw[:, (b % GRP) * 132:
                                       (b % GRP) * 132 + 132]
                            T = int(Tu[P][b])
                            if T == 0:
                                nc.vector.tensor_copy(out=osb, in_=zrow[:])
                            else:
                                lo, hi = int(cum[P, b]), int(cum[P, b + 1])
                                need(hi)
                                spans = []
                                t = lo
                                while t < hi:
                                    g = t // GT
                                    tl = t % GT
                                    n = min(GT - tl, hi - t)
                                    spans.append((cache[g], tl, n, t - lo))
                                    t += n
                                msf = pmsg.tile([128, Tmax * 132], BF16, tag="ms")
                                mv = msf[:, 0:T * 132].rearrange(
                                    "p (t c) -> p t c", c=132)
                                lgf = pmsg.tile([128, Tmax * 4], BF16, tag="lg")
                                lg = lgf[:, 0:T * 4]
                                lv = lg.rearrange("p (t h) -> p t h", h=4)
                                abv = aibs[P][:].rearrange("p (b h) -> p b h",
                                                           h=4)
                                for (hv, tl, n, o) in spans:
                                    nc.vector.tensor_tensor(
                                        out=lv[:, o:o + n, :],
                                        in0=hv[:, tl:tl + n,
                                               128 + 4 * m:132 + 4 * m],
                                        in1=abv[:, b:b + 1, :].to_broadcast(
                                            (128, n, 4)),
                                        op=OP.add)
                                lrf = pmsg.tile([128, Tmax * 4], BF16, tag="lr")
                                lrt = lrf[:, 0:T * 4]
                                nc.scalar.activation(lrt, lg, AF.Prelu,
                                                     alpha=0.2)
                                sxf = psxp.tile([128, Tmax * 128], BF16, tag="sx")
                                xv = sxf[:, 0:T * 128].rearrange(
                                    "p (t c) -> p t c", c=128)
                                nc.scalar.activation(
                                    xv.rearrange("p t (h c) -> p (t h) c", c=32),
                                    lrt.unsqueeze(2).to_broadcast(
                                        (128, T * 4, 32)),
                                    AF.Exp)
                                nc.scalar.activation(
                                    mv[:, :, 128:132],
                                    lrt.rearrange("p (t h) -> p t h", h=4),
                                    AF.Exp)
                                for (hv, tl, n, o) in spans:
                                    nc.vector.tensor_tensor(
                                        out=mv[:, o:o + n, 0:128],
                                        in0=hv[:, tl:tl + n, 0:128],
                                        in1=xv[:, o:o + n, :],
                                        op=OP.mult)
                                n = T
                                while n > 2:
                                    hh = n // 2
                                    nc.vector.tensor_tensor(
                                        out=msf[:, 0:hh * 132],
                                        in0=msf[:, 0:hh * 132],
                                        in1=msf[:, (n - hh) * 132:n * 132],
                                        op=OP.add)
                                    n -= hh
                                if n == 2:
                                    nc.vector.tensor_tensor(
                                        out=osb, in0=msf[:, 0:132],
                                        in1=msf[:, 132:264], op=OP.add)
                                else:
                                    nc.vector.tensor_copy(out=osb,
                                                          in_=msf[:, 0:132])
                            if b % GRP == GRP - 1 or b == NB - 1:
                                b0 = (b // GRP) * GRP
                                nb2 = b - b0 + 1
                                nc.sync.dma_start(
                                    out=ERAW[ds(P * CORE_N + b0 * 128,
                                                nb2 * 128),
                                             0:132].rearrange(
                                                 "(g p) c -> p g c", p=128),
                                    in_=osbw[:, 0:nb2 * 132].rearrange(
                                        "p (g c) -> p g c", c=132))

                        # realign + merge this metapath once both chunk
                        # passes are done (the last metapath interleaves
                        # with its own block loop above)
                        if P % 2 == 1:
                            ita = prl.tile([128, NCHO * GW], I16, tag="ita",
                                           name="ita")
                            nc.sync.dma_start(
                                out=ita[:],
                                in_=IDXW[:, ds(aln_off + (P - 1) * NCHO * GW,
                                               NCHO * GW)])
                            itb = prl.tile([128, NCHO * GW], I16, tag="itb",
                                           name="itb")
                            nc.sync.dma_start(
                                out=itb[:],
                                in_=IDXW[:, ds(aln_off + P * NCHO * GW,
                                               NCHO * GW)])
                            for g in range(NCHO):
                                emit_rl_chunk(P // 2, P, g, ita, itb)
                                if P == NP - 1:
                                    for b3 in range(g * GT,
                                                    min((g + 1) * GT, NB)):
                                        emit_s3(b3)

    nc.compile()
    return nc


def kernel(feats, edge_index, W, b, attn, rel_attn_l, rel_attn_r,
           rel_attn_bias, _trace=False):
    plan, in_maps = _prep_host(feats, edge_index, W, attn, rel_attn_l,
                               rel_attn_r, rel_attn_bias)
    key = tuple(plan["Tu"].ravel())
    if key not in _CACHE:
        _CACHE.clear()
        _CACHE[key] = _build(plan)
    nc = _CACHE[key]
    res = run_bass_kernel_spmd(nc, in_maps, core_ids=list(range(NCORES)),
                               trace=_trace)
    parts = []
    for k in range(NCORES):
        rows = min(CORE_N, N - CORE_N * k)
        parts.append(np.asarray(res.results[k]["OUT"][:rows],
                                dtype=np.float32))
    out = np.concatenate(parts, axis=0)
    if _trace:
        kernel._last_exec_ns = res.exec_time_ns
    return out

